# revision 31
# baseline (speedup 1.0000x reference)
"""Trainium2 Bass kernel for nn_NonLinearOp (integrate-and-fire scan).

Math per element x[t,b,d] (scalar v):
  h[n] = x*W1[n] + b1[n]            (n = 64 neurons)
  scan over t: v += h_t; spike = thr*(v>=thr); v -= spike
  y[t,b,d] = sum_n spike[t,b,d,n]*W2[n] + b2

Device mapping (8 cores, D sharded 4096 -> 512 per core):
  - partitions = 64 neurons x 2 b-values (128); free dim = 512 d-cols.
  - TensorE accumulates v (normalized by 1/thr) in PSUM via K=15 fp16
    triple-split matmuls (products fp32-exact), one per T-step per pair.
  - spike indicator:
      strat 'A': DVE tensor_scalar is_ge (exact, incl. v==thr)
      strat 'B': ScalarE Sign(v-1) -> s' in {-1,0,1}; affine terms folded
                 into the W1/b1 weight rows, W2 scaling, and b2.
  - soft reset: DVE subtract (A) / DVE fused stt or PE -0.5*Id matmul (B)
  - y: TensorE K=128 matmul with fp16 W2, M=32 slots (32-aligned, zero-pad
    cols) accumulated per pair in a 2-bank PSUM tile; evac + b2 on DVE.
  - software-pipelined wavefront: 4 pairs per wave step-interleaved, y
    matmuls of wave w-1 run inside wave w's scan slots.
"""

import numpy as np

import concourse.bass as bass
import concourse.bacc as bacc
import concourse.mybir as mybir
import concourse.tile as tile
from concourse.bass_utils import run_bass_kernel_spmd

F16 = mybir.dt.float16
F32 = mybir.dt.float32

T, B, D, N = 8, 16, 4096, 64
NCORES = 8
DLOC = D // NCORES          # 512
PAIRS = 8                   # b-blocks of 2 -> groups A/B = single b each
FREE = DLOC                 # 512 free elements per pair tile

# ---- tuning knobs ----
STRATS = ("B",) * 8             # per-pair: 'A' (DVE is_ge+sub) or 'B' (ACT Sign)
INJ_DVE_PAIRS = frozenset({0, 1, 2, 3, 4, 5})  # B-pairs injecting on DVE
W2_SPLIT = False                # hi/lo split of W2 contraction
WAVE = 6                        # pairs per pipeline wave

_prog_cache = {}
TRACE = False          # set by test.py; harness leaves it False
LAST_EXEC_NS = None    # filled from the NTFF profile when TRACE


def _ensure_ntff_hook():
    """The container image's antenv lacks axon_hooks; synthesize it so
    run_bass_kernel_spmd(trace=True) can capture NTFF profiles."""
    import sys as _sys
    if "antenv.axon_hooks" in _sys.modules:
        return
    import contextlib
    import ctypes
    import types

    so_path = "/opt/axon/libaxon_pjrt.so"
    try:
        lib = ctypes.CDLL(so_path)
    except OSError:
        return
    if not hasattr(lib, "axon_start_nrt_profile"):
        return
    lib.axon_start_nrt_profile.argtypes = [ctypes.POINTER(ctypes.c_int64),
                                           ctypes.c_size_t]
    lib.axon_start_nrt_profile.restype = ctypes.c_int64
    lib.axon_stop_nrt_profile.argtypes = [ctypes.c_char_p]
    lib.axon_stop_nrt_profile.restype = ctypes.c_int64

    @contextlib.contextmanager
    def _hook(output_dir, device_ids):
        import jax
        jax.devices()
        if device_ids:
            ids = (ctypes.c_int64 * len(device_ids))(*device_ids)
            rc = lib.axon_start_nrt_profile(ids, len(device_ids))
        else:
            rc = lib.axon_start_nrt_profile(None, 0)
        if rc != 0:
            raise RuntimeError(f"axon_start_nrt_profile rc={rc}")
        try:
            yield
        finally:
            n = lib.axon_stop_nrt_profile(str(output_dir).encode())
            print(f"ntff profile: {n} file(s) written to {output_dir}")

    mod = types.ModuleType("antenv.axon_hooks")
    mod.get_axon_ntff_profile_hook = lambda: _hook
    mod.set_axon_ntff_profile_hook = lambda h: None
    _sys.modules["antenv.axon_hooks"] = mod


def _dedup_ldweights(nc):
    """Drop PE-stream-consecutive InstLdweights with identical weight APs
    (PE weight regs persist across matmuls); only sem-free dups removed."""
    removed = 0
    for blk in nc.m.functions[0].blocks:
        keep = []
        last_sig = None
        for ins in blk.instructions:
            if getattr(ins, "engine", None) == mybir.EngineType.PE:
                nm = type(ins).__name__
                if nm == "InstLdweights":
                    sig = repr(ins.ins[0])
                    si = ins.sync_info
                    clean = si is None or (not si.on_wait and not si.on_update)
                    if sig == last_sig and clean:
                        removed += 1
                        continue
                    last_sig = sig
                elif nm not in ("InstMatmult", "InstNop", "InstEventSemaphore"):
                    last_sig = None
            keep.append(ins)
        blk.instructions[:] = keep
    return removed


def _split3(v):
    """fp32 -> 3x fp16 (hi, mid, lo) with hi+mid+lo ~ v to ~2^-33."""
    v = v.astype(np.float32)
    hi = v.astype(np.float16)
    r = v - hi.astype(np.float32)
    mid = r.astype(np.float16)
    lo = (r - mid.astype(np.float32)).astype(np.float16)
    return hi, mid, lo


def _build_program():
    """One SPMD program; all weights/data arrive as ExternalInputs."""
    nc = bacc.Bacc(None, target_bir_lowering=False)
    xstage = nc.dram_tensor("xstage", [PAIRS * 15, T * FREE], F16,
                            kind="ExternalInput")
    wh = nc.dram_tensor("wh", [30, 128], F16, kind="ExternalInput")
    wy = nc.dram_tensor("wy", [128, 128], F16, kind="ExternalInput")
    idm = nc.dram_tensor("idm", [128, 128], F16, kind="ExternalInput")
    b2c = nc.dram_tensor("b2c", [128, 4], F32, kind="ExternalInput")
    yout = nc.dram_tensor("y", [T * B, DLOC], F32, kind="ExternalOutput")

    with tile.TileContext(nc) as tc:
        with (
            tc.tile_pool(name="const", bufs=1) as constp,
            tc.tile_pool(name="stage", bufs=2 * WAVE) as stagep,
            tc.tile_pool(name="spool", bufs=48) as spool,
            tc.tile_pool(name="ysb", bufs=2) as ysbp,
            tc.tile_pool(name="vps", bufs=WAVE, space="PSUM") as vpool,
            tc.tile_pool(name="yps", bufs=1, space="PSUM") as ypool,
        ):
            wh_t = constp.tile([15, 128], F16)
            nc.sync.dma_start(out=wh_t[:], in_=wh[0:15, :])
            wh_r = constp.tile([15, 128], F16)
            nc.sync.dma_start(out=wh_r[:], in_=wh[15:30, :])
            wy_t = constp.tile([128, 128], F16)
            nc.sync.dma_start(out=wy_t[:], in_=wy[:])
            b2_t = constp.tile([128, 4], F32)
            nc.sync.dma_start(out=b2_t[:], in_=b2c[:])
            id_t = constp.tile([128, 128], F16)
            nc.sync.dma_start(out=id_t[:], in_=idm[:])

            n_waves = PAIRS // WAVE
            s_tiles = {}
            v_tiles = {}
            stage_tiles = {}
            ysl_tiles = {}

            def stage_load(p):
                stage = stagep.tile([15, T, FREE], F16, name=f"stage{p}",
                                    tag="stage")
                stage_tiles[p] = stage
                (nc.sync if p % 2 == 0 else nc.gpsimd).dma_start(
                    out=stage[:, :, :],
                    in_=xstage[15 * p:15 * (p + 1), :].rearrange(
                        "r (t f) -> r t f", t=T),
                )

            def emit_y_phase(p, thalf):
                """y matmuls for pair p, t in [4*thalf, 4*thalf+4)."""
                strat = STRATS[p % len(STRATS)]
                wyo = 64 if strat == "B" else 0
                if thalf == 0:
                    ysl_tiles[p] = ypool.tile([128, 2 * FREE], F32,
                                              name=f"ysl{p}", tag="ysl")
                ysl = ysl_tiles[p]
                for t in range(4 * thalf, 4 * thalf + 4):
                    s = s_tiles.pop((p, t))
                    prow = 32 * (t % 4)
                    cbase = (t // 4) * FREE
                    outsl = ysl[prow:prow + 32, cbase:cbase + FREE]
                    tp = (0, prow)
                    if W2_SPLIT:
                        nc.tensor.matmul(outsl, lhsT=wy_t[:, wyo:wyo + 32],
                                         rhs=s[:, :], start=True, stop=False,
                                         tile_position=tp)
                        nc.tensor.matmul(outsl, lhsT=wy_t[:, wyo + 32:wyo + 64],
                                         rhs=s[:, :], start=False, stop=True,
                                         tile_position=tp)
                    else:
                        nc.tensor.matmul(outsl, lhsT=wy_t[:, wyo:wyo + 32],
                                         rhs=s[:, :], start=True, stop=True,
                                         tile_position=tp)
                if thalf == 1:
                    emit_evac(p, ysl)

            def emit_evac(p, ysl):
                strat = STRATS[p % len(STRATS)]
                y_sb = ysbp.tile([128, 2 * FREE], F32, name=f"ysb{p}",
                                 tag="ysb")
                bcol = 0 if strat == "A" else 1
                nc.vector.tensor_scalar(
                    out=y_sb[:], in0=ysl[:], scalar1=b2_t[:, bcol:bcol + 1],
                    scalar2=None, op0=mybir.AluOpType.add,
                )
                ypext = y_sb.ap[0][0]
                for k in range(4):
                    # t = k and t = k+4 share src partitions {32k, 32k+1}
                    src_ap = bass.AP(
                        tensor=y_sb.tensor,
                        offset=y_sb.offset + 32 * k * ypext,
                        ap=[[ypext, 2], [FREE, 2], [1, FREE]],
                    )
                    dst_ap = bass.AP(
                        tensor=yout,
                        offset=(k * B + 2 * p) * DLOC,
                        ap=[[DLOC, 2], [4 * B * DLOC, 2], [1, DLOC]],
                    )
                    eng = nc.gpsimd if k % 2 == 0 else nc.sync
                    eng.dma_start(out=dst_ap, in_=src_ap)

            for p in range(min(WAVE, PAIRS)):
                stage_load(p)

            yq = []           # ready y-units (pair, thalf)
            loaded = min(WAVE, PAIRS)
            waves = []
            q = 0
            while q < PAIRS:
                waves.append(list(range(q, min(q + WAVE, PAIRS))))
                q += WAVE
            for wave in waves:
                for t in range(T):
                    # drain ready y work into this slot (up to 2 units)
                    for _ in range(2):
                        if yq:
                            pp, hh = yq.pop(0)
                            emit_y_phase(pp, hh)
                    # --- h accumulate ---
                    for p in wave:
                        strat = STRATS[p % len(STRATS)]
                        if t == 0:
                            v_tiles[p] = vpool.tile([128, FREE], F32,
                                                    name=f"v{p}", tag="v")
                        whv = wh_t[:] if (strat == "A" or t == 0) else wh_r[:]
                        nc.tensor.matmul(
                            v_tiles[p][:, :], lhsT=whv,
                            rhs=stage_tiles[p][:, t, :],
                            start=(t == 0), stop=True, skip_group_check=True,
                        )
                    # --- spike indicator ---
                    for p in wave:
                        strat = STRATS[p % len(STRATS)]
                        s = spool.tile([128, FREE], F16, name=f"s{p}_{t}",
                                       tag="s")
                        s_tiles[(p, t)] = s
                        v = v_tiles[p]
                        if strat == "A":
                            nc.vector.tensor_scalar(
                                out=s[:], in0=v[:], scalar1=1.0, scalar2=None,
                                op0=mybir.AluOpType.is_ge,
                            )
                        else:
                            nc.scalar.activation(
                                out=s[:], in_=v[:],
                                func=mybir.ActivationFunctionType.Sign,
                                bias=b2_t[:, 2:3], scale=1.0,
                            )
                        if t == 3:
                            yq.append((p, 0))
                        elif t == T - 1:
                            yq.append((p, 1))
                    # --- reset inject ---
                    if t < T - 1:
                        for p in wave:
                            strat = STRATS[p % len(STRATS)]
                            s, v = s_tiles[(p, t)], v_tiles[p]
                            if strat == "A":
                                nc.vector.tensor_tensor(
                                    out=v[:], in0=v[:], in1=s[:],
                                    op=mybir.AluOpType.subtract,
                                )
                            elif p in INJ_DVE_PAIRS:
                                nc.vector.scalar_tensor_tensor(
                                    out=v[:], in0=s[:], scalar=-0.5, in1=v[:],
                                    op0=mybir.AluOpType.mult,
                                    op1=mybir.AluOpType.add,
                                )
                            else:
                                nc.tensor.matmul(
                                    v[:, :], lhsT=id_t[:, :], rhs=s[:, :],
                                    start=False, stop=True,
                                    skip_group_check=True,
                                )
                    if loaded < PAIRS:
                        stage_load(loaded)
                        loaded += 1
            # tail: drain remaining y work
            for pp, hh in yq:
                emit_y_phase(pp, hh)
    nc.compile()
    return nc


def _prep_inputs(x, W1, b1, W2, b2, thr):
    a = W1[:, 0].astype(np.float32) / thr
    c = b1.astype(np.float32) / thr
    ah, am, al = _split3(a)
    w2p = thr * W2[0].astype(np.float32)

    wh = np.zeros((30, 128), np.float16)
    for base, cvals in ((0, c), (15, c - 0.5)):
        ch, cm, cl = _split3(cvals)
        for g in range(2):
            cols = slice(64 * g, 64 * (g + 1))
            rows = base + 6 * g
            wh[rows + 0, cols] = ah   # pairs x_hi
            wh[rows + 1, cols] = am   # pairs x_hi
            wh[rows + 2, cols] = al   # pairs x_hi
            wh[rows + 3, cols] = ah   # pairs x_mid
            wh[rows + 4, cols] = am   # pairs x_mid
            wh[rows + 5, cols] = ah   # pairs x_lo
        wh[base + 12, :64] = wh[base + 12, 64:] = ch
        wh[base + 13, :64] = wh[base + 13, 64:] = cm
        wh[base + 14, :64] = wh[base + 14, 64:] = cl

    wy = np.zeros((128, 128), np.float16)
    for varbase, scale in ((0, 1.0), (64, 0.5)):
        vv = (scale * w2p).astype(np.float32)
        vh = vv.astype(np.float16)
        vl = (vv - vh.astype(np.float32)).astype(np.float16)
        for g in range(2):
            rows = slice(64 * g, 64 * (g + 1))
            wy[rows, varbase + g] = vh          # hi block cols 0,1
            wy[rows, varbase + 32 + g] = vl     # lo block cols 32,33
    idm = (-0.5 * np.eye(128, dtype=np.float32)).astype(np.float16)
    b2c = np.zeros((128, 4), np.float32)
    b2c[:, 0] = np.float32(b2[0])
    b2c[:, 1] = np.float32(b2[0] + 0.5 * w2p.sum(dtype=np.float32))
    b2c[:, 2] = np.float32(-1.0)
    return wh, wy, idm, b2c


def _build_xstage(x):
    """Per-core staged x layout: [NCORES][PAIRS*15, T*FREE] fp16.

    rows 0-2: x_hi grpA (x3), 3-4: x_mid A, 5: x_lo A, 6-11: same grpB,
    12-14: ones.  free = t*FREE + d,  b = 2p + g.
    """
    xh, xm_, xl_ = _split3(x)                       # [T*B, D]
    out = np.empty((NCORES, PAIRS * 15, T * FREE), np.float16)
    rowmap = [(0, 0), (0, 0), (0, 0), (1, 0), (1, 0), (2, 0),
              (0, 1), (0, 1), (0, 1), (1, 1), (1, 1), (2, 1)]
    parts = (xh, xm_, xl_)
    for core in range(NCORES):
        dsl = slice(core * DLOC, (core + 1) * DLOC)
        for p in range(PAIRS):
            for r, (pi, g) in enumerate(rowmap):
                arr = parts[pi].reshape(T, B, D)[:, 2 * p + g, dsl]
                out[core, 15 * p + r] = arr.reshape(-1)
            out[core, 15 * p + 12:15 * p + 15] = np.float16(1.0)
    return out


def kernel(x, W1, b1, W2, b2, threshold, T: int = T, **_unused):
    x = np.asarray(x, np.float32)
    W1 = np.asarray(W1, np.float32)
    b1 = np.asarray(b1, np.float32)
    W2 = np.asarray(W2, np.float32)
    b2 = np.atleast_1d(np.asarray(b2, np.float32))
    thr = float(np.asarray(threshold))
    assert x.shape == (128, 4096), x.shape

    key = (STRATS, tuple(sorted(INJ_DVE_PAIRS)), W2_SPLIT, WAVE)
    if key not in _prog_cache:
        _prog_cache[key] = _build_program()
    nc = _prog_cache[key]

    wh, wy, idm, b2c = _prep_inputs(x, W1, b1, W2, b2, thr)
    xstage = _build_xstage(x)

    in_maps = []
    for core in range(NCORES):
        in_maps.append({
            "xstage": xstage[core],
            "wh": wh,
            "wy": wy,
            "idm": idm,
            "b2c": b2c,
        })
    global LAST_EXEC_NS
    kw = {}
    if TRACE:
        _ensure_ntff_hook()
        kw = dict(trace=True, trace_cores=[0])
    res = run_bass_kernel_spmd(nc, in_maps, core_ids=list(range(NCORES)), **kw)
    if TRACE:
        LAST_EXEC_NS = res.exec_time_ns
    out = np.concatenate([res.results[i]["y"] for i in range(NCORES)], axis=1)
    return out.astype(np.float32)


# revision 32
# speedup vs baseline: 1.0435x; 1.0435x over previous
"""Trainium2 Bass kernel for nn_NonLinearOp (integrate-and-fire scan).

Math per element x[t,b,d] (scalar v):
  h[n] = x*W1[n] + b1[n]            (n = 64 neurons)
  scan over t: v += h_t; spike = thr*(v>=thr); v -= spike
  y[t,b,d] = sum_n spike[t,b,d,n]*W2[n] + b2

Device mapping (8 cores, D sharded 4096 -> 512 per core):
  - partitions = 64 neurons x 2 b-values (128); free dim = 512 d-cols.
  - TensorE accumulates v (normalized by 1/thr) in PSUM via K=15 fp16
    triple-split matmuls (products fp32-exact), one per T-step per pair.
  - spike indicator:
      strat 'A': DVE tensor_scalar is_ge (exact, incl. v==thr)
      strat 'B': ScalarE Sign(v-1) -> s' in {-1,0,1}; affine terms folded
                 into the W1/b1 weight rows, W2 scaling, and b2.
  - soft reset: DVE subtract (A) / DVE fused stt or PE -0.5*Id matmul (B)
  - y: TensorE K=128 matmul with fp16 W2, M=32 slots (32-aligned, zero-pad
    cols) accumulated per pair in a 2-bank PSUM tile; evac + b2 on DVE.
  - software-pipelined wavefront: 4 pairs per wave step-interleaved, y
    matmuls of wave w-1 run inside wave w's scan slots.
"""

import numpy as np

import concourse.bass as bass
import concourse.bacc as bacc
import concourse.mybir as mybir
import concourse.tile as tile
from concourse.bass_utils import run_bass_kernel_spmd

F16 = mybir.dt.float16
F32 = mybir.dt.float32

T, B, D, N = 8, 16, 4096, 64
NCORES = 8
DLOC = D // NCORES          # 512
PAIRS = 8                   # b-blocks of 2 -> groups A/B = single b each
FREE = DLOC                 # 512 free elements per pair tile

# ---- tuning knobs ----
STRATS = ("B",) * 8             # per-pair: 'A' (DVE is_ge+sub) or 'B' (ACT Sign)
INJ_DVE_PAIRS = frozenset(range(8))  # B-pairs injecting on DVE
W2_SPLIT = False                # hi/lo split of W2 contraction
WAVE = 6                        # pairs per pipeline wave

_prog_cache = {}
TRACE = False          # set by test.py; harness leaves it False
LAST_EXEC_NS = None    # filled from the NTFF profile when TRACE


def _ensure_ntff_hook():
    """The container image's antenv lacks axon_hooks; synthesize it so
    run_bass_kernel_spmd(trace=True) can capture NTFF profiles."""
    import sys as _sys
    if "antenv.axon_hooks" in _sys.modules:
        return
    import contextlib
    import ctypes
    import types

    so_path = "/opt/axon/libaxon_pjrt.so"
    try:
        lib = ctypes.CDLL(so_path)
    except OSError:
        return
    if not hasattr(lib, "axon_start_nrt_profile"):
        return
    lib.axon_start_nrt_profile.argtypes = [ctypes.POINTER(ctypes.c_int64),
                                           ctypes.c_size_t]
    lib.axon_start_nrt_profile.restype = ctypes.c_int64
    lib.axon_stop_nrt_profile.argtypes = [ctypes.c_char_p]
    lib.axon_stop_nrt_profile.restype = ctypes.c_int64

    @contextlib.contextmanager
    def _hook(output_dir, device_ids):
        import jax
        jax.devices()
        if device_ids:
            ids = (ctypes.c_int64 * len(device_ids))(*device_ids)
            rc = lib.axon_start_nrt_profile(ids, len(device_ids))
        else:
            rc = lib.axon_start_nrt_profile(None, 0)
        if rc != 0:
            raise RuntimeError(f"axon_start_nrt_profile rc={rc}")
        try:
            yield
        finally:
            n = lib.axon_stop_nrt_profile(str(output_dir).encode())
            print(f"ntff profile: {n} file(s) written to {output_dir}")

    mod = types.ModuleType("antenv.axon_hooks")
    mod.get_axon_ntff_profile_hook = lambda: _hook
    mod.set_axon_ntff_profile_hook = lambda h: None
    _sys.modules["antenv.axon_hooks"] = mod


def _dedup_ldweights(nc):
    """Drop PE-stream-consecutive InstLdweights with identical weight APs
    (PE weight regs persist across matmuls); only sem-free dups removed."""
    removed = 0
    for blk in nc.m.functions[0].blocks:
        keep = []
        last_sig = None
        for ins in blk.instructions:
            if getattr(ins, "engine", None) == mybir.EngineType.PE:
                nm = type(ins).__name__
                if nm == "InstLdweights":
                    sig = repr(ins.ins[0])
                    si = ins.sync_info
                    clean = si is None or (not si.on_wait and not si.on_update)
                    if sig == last_sig and clean:
                        removed += 1
                        continue
                    last_sig = sig
                elif nm not in ("InstMatmult", "InstNop", "InstEventSemaphore"):
                    last_sig = None
            keep.append(ins)
        blk.instructions[:] = keep
    return removed


def _split3(v):
    """fp32 -> 3x fp16 (hi, mid, lo) with hi+mid+lo ~ v to ~2^-33."""
    v = v.astype(np.float32)
    hi = v.astype(np.float16)
    r = v - hi.astype(np.float32)
    mid = r.astype(np.float16)
    lo = (r - mid.astype(np.float32)).astype(np.float16)
    return hi, mid, lo


def _build_program():
    """One SPMD program; all weights/data arrive as ExternalInputs."""
    nc = bacc.Bacc(None, target_bir_lowering=False)
    xstage = nc.dram_tensor("xstage", [PAIRS * 15, T * FREE], F16,
                            kind="ExternalInput")
    wh = nc.dram_tensor("wh", [30, 128], F16, kind="ExternalInput")
    wy = nc.dram_tensor("wy", [128, 128], F16, kind="ExternalInput")
    idm = nc.dram_tensor("idm", [128, 128], F16, kind="ExternalInput")
    b2c = nc.dram_tensor("b2c", [128, 4], F32, kind="ExternalInput")
    yout = nc.dram_tensor("y", [T * B, DLOC], F32, kind="ExternalOutput")

    with tile.TileContext(nc) as tc:
        with (
            tc.tile_pool(name="const", bufs=1) as constp,
            tc.tile_pool(name="stage", bufs=2 * WAVE) as stagep,
            tc.tile_pool(name="spool", bufs=48) as spool,
            tc.tile_pool(name="ysb", bufs=2) as ysbp,
            tc.tile_pool(name="vps", bufs=WAVE, space="PSUM") as vpool,
            tc.tile_pool(name="yps", bufs=1, space="PSUM") as ypool,
        ):
            wh_t = constp.tile([15, 128], F16)
            nc.sync.dma_start(out=wh_t[:], in_=wh[0:15, :])
            wh_r = constp.tile([15, 128], F16)
            nc.sync.dma_start(out=wh_r[:], in_=wh[15:30, :])
            wy_t = constp.tile([128, 128], F16)
            nc.sync.dma_start(out=wy_t[:], in_=wy[:])
            b2_t = constp.tile([128, 4], F32)
            nc.sync.dma_start(out=b2_t[:], in_=b2c[:])
            id_t = constp.tile([128, 128], F16)
            nc.sync.dma_start(out=id_t[:], in_=idm[:])

            n_waves = PAIRS // WAVE
            s_tiles = {}
            v_tiles = {}
            stage_tiles = {}
            ysl_tiles = {}

            def stage_load(p):
                stage = stagep.tile([15, T, FREE], F16, name=f"stage{p}",
                                    tag="stage")
                stage_tiles[p] = stage
                (nc.sync if p % 2 == 0 else nc.gpsimd).dma_start(
                    out=stage[:, :, :],
                    in_=xstage[15 * p:15 * (p + 1), :].rearrange(
                        "r (t f) -> r t f", t=T),
                )

            def emit_y_phase(p, thalf):
                """y matmuls for pair p, t in [4*thalf, 4*thalf+4)."""
                strat = STRATS[p % len(STRATS)]
                wyo = 64 if strat == "B" else 0
                if thalf == 0:
                    ysl_tiles[p] = ypool.tile([128, 2 * FREE], F32,
                                              name=f"ysl{p}", tag="ysl")
                ysl = ysl_tiles[p]
                for t in range(4 * thalf, 4 * thalf + 4):
                    s = s_tiles.pop((p, t))
                    prow = 32 * (t % 4)
                    cbase = (t // 4) * FREE
                    outsl = ysl[prow:prow + 32, cbase:cbase + FREE]
                    tp = (0, prow)
                    if W2_SPLIT:
                        nc.tensor.matmul(outsl, lhsT=wy_t[:, wyo:wyo + 32],
                                         rhs=s[:, :], start=True, stop=False,
                                         tile_position=tp)
                        nc.tensor.matmul(outsl, lhsT=wy_t[:, wyo + 32:wyo + 64],
                                         rhs=s[:, :], start=False, stop=True,
                                         tile_position=tp)
                    else:
                        nc.tensor.matmul(outsl, lhsT=wy_t[:, wyo:wyo + 32],
                                         rhs=s[:, :], start=True, stop=True,
                                         tile_position=tp)
                if thalf == 1:
                    emit_evac(p, ysl)

            def emit_evac(p, ysl):
                strat = STRATS[p % len(STRATS)]
                y_sb = ysbp.tile([128, 2 * FREE], F32, name=f"ysb{p}",
                                 tag="ysb")
                bcol = 0 if strat == "A" else 1
                nc.vector.tensor_scalar(
                    out=y_sb[:], in0=ysl[:], scalar1=b2_t[:, bcol:bcol + 1],
                    scalar2=None, op0=mybir.AluOpType.add,
                )
                ypext = y_sb.ap[0][0]
                for k in range(4):
                    # t = k and t = k+4 share src partitions {32k, 32k+1}
                    src_ap = bass.AP(
                        tensor=y_sb.tensor,
                        offset=y_sb.offset + 32 * k * ypext,
                        ap=[[ypext, 2], [FREE, 2], [1, FREE]],
                    )
                    dst_ap = bass.AP(
                        tensor=yout,
                        offset=(k * B + 2 * p) * DLOC,
                        ap=[[DLOC, 2], [4 * B * DLOC, 2], [1, DLOC]],
                    )
                    eng = nc.gpsimd if k % 2 == 0 else nc.sync
                    eng.dma_start(out=dst_ap, in_=src_ap)

            for p in range(min(WAVE, PAIRS)):
                stage_load(p)

            yq = []           # ready y-units (pair, thalf)
            loaded = min(WAVE, PAIRS)
            waves = []
            q = 0
            while q < PAIRS:
                waves.append(list(range(q, min(q + WAVE, PAIRS))))
                q += WAVE
            for wave in waves:
                for t in range(T):
                    # drain ready y work into this slot (up to 2 units)
                    for _ in range(2):
                        if yq:
                            pp, hh = yq.pop(0)
                            emit_y_phase(pp, hh)
                    # --- h accumulate ---
                    for p in wave:
                        strat = STRATS[p % len(STRATS)]
                        if t == 0:
                            v_tiles[p] = vpool.tile([128, FREE], F32,
                                                    name=f"v{p}", tag="v")
                        whv = wh_t[:] if (strat == "A" or t == 0) else wh_r[:]
                        nc.tensor.matmul(
                            v_tiles[p][:, :], lhsT=whv,
                            rhs=stage_tiles[p][:, t, :],
                            start=(t == 0), stop=True, skip_group_check=True,
                        )
                    # --- spike indicator ---
                    for p in wave:
                        strat = STRATS[p % len(STRATS)]
                        s = spool.tile([128, FREE], F16, name=f"s{p}_{t}",
                                       tag="s")
                        s_tiles[(p, t)] = s
                        v = v_tiles[p]
                        if strat == "A":
                            nc.vector.tensor_scalar(
                                out=s[:], in0=v[:], scalar1=1.0, scalar2=None,
                                op0=mybir.AluOpType.is_ge,
                            )
                        else:
                            nc.scalar.activation(
                                out=s[:], in_=v[:],
                                func=mybir.ActivationFunctionType.Sign,
                                bias=b2_t[:, 2:3], scale=1.0,
                            )
                        if t == 3:
                            yq.append((p, 0))
                        elif t == T - 1:
                            yq.append((p, 1))
                    # --- reset inject ---
                    if t < T - 1:
                        for p in wave:
                            strat = STRATS[p % len(STRATS)]
                            s, v = s_tiles[(p, t)], v_tiles[p]
                            if strat == "A":
                                nc.vector.tensor_tensor(
                                    out=v[:], in0=v[:], in1=s[:],
                                    op=mybir.AluOpType.subtract,
                                )
                            elif p in INJ_DVE_PAIRS:
                                nc.vector.scalar_tensor_tensor(
                                    out=v[:], in0=s[:], scalar=-0.5, in1=v[:],
                                    op0=mybir.AluOpType.mult,
                                    op1=mybir.AluOpType.add,
                                )
                            else:
                                nc.tensor.matmul(
                                    v[:, :], lhsT=id_t[:, :], rhs=s[:, :],
                                    start=False, stop=True,
                                    skip_group_check=True,
                                )
                    if loaded < PAIRS:
                        stage_load(loaded)
                        loaded += 1
            # tail: drain remaining y work
            for pp, hh in yq:
                emit_y_phase(pp, hh)
    nc.compile()
    return nc


def _prep_inputs(x, W1, b1, W2, b2, thr):
    a = W1[:, 0].astype(np.float32) / thr
    c = b1.astype(np.float32) / thr
    ah, am, al = _split3(a)
    w2p = thr * W2[0].astype(np.float32)

    wh = np.zeros((30, 128), np.float16)
    for base, cvals in ((0, c), (15, c - 0.5)):
        ch, cm, cl = _split3(cvals)
        for g in range(2):
            cols = slice(64 * g, 64 * (g + 1))
            rows = base + 6 * g
            wh[rows + 0, cols] = ah   # pairs x_hi
            wh[rows + 1, cols] = am   # pairs x_hi
            wh[rows + 2, cols] = al   # pairs x_hi
            wh[rows + 3, cols] = ah   # pairs x_mid
            wh[rows + 4, cols] = am   # pairs x_mid
            wh[rows + 5, cols] = ah   # pairs x_lo
        wh[base + 12, :64] = wh[base + 12, 64:] = ch
        wh[base + 13, :64] = wh[base + 13, 64:] = cm
        wh[base + 14, :64] = wh[base + 14, 64:] = cl

    wy = np.zeros((128, 128), np.float16)
    for varbase, scale in ((0, 1.0), (64, 0.5)):
        vv = (scale * w2p).astype(np.float32)
        vh = vv.astype(np.float16)
        vl = (vv - vh.astype(np.float32)).astype(np.float16)
        for g in range(2):
            rows = slice(64 * g, 64 * (g + 1))
            wy[rows, varbase + g] = vh          # hi block cols 0,1
            wy[rows, varbase + 32 + g] = vl     # lo block cols 32,33
    idm = (-0.5 * np.eye(128, dtype=np.float32)).astype(np.float16)
    b2c = np.zeros((128, 4), np.float32)
    b2c[:, 0] = np.float32(b2[0])
    b2c[:, 1] = np.float32(b2[0] + 0.5 * w2p.sum(dtype=np.float32))
    b2c[:, 2] = np.float32(-1.0)
    return wh, wy, idm, b2c


def _build_xstage(x):
    """Per-core staged x layout: [NCORES][PAIRS*15, T*FREE] fp16.

    rows 0-2: x_hi grpA (x3), 3-4: x_mid A, 5: x_lo A, 6-11: same grpB,
    12-14: ones.  free = t*FREE + d,  b = 2p + g.
    """
    xh, xm_, xl_ = _split3(x)                       # [T*B, D]
    out = np.empty((NCORES, PAIRS * 15, T * FREE), np.float16)
    rowmap = [(0, 0), (0, 0), (0, 0), (1, 0), (1, 0), (2, 0),
              (0, 1), (0, 1), (0, 1), (1, 1), (1, 1), (2, 1)]
    parts = (xh, xm_, xl_)
    for core in range(NCORES):
        dsl = slice(core * DLOC, (core + 1) * DLOC)
        for p in range(PAIRS):
            for r, (pi, g) in enumerate(rowmap):
                arr = parts[pi].reshape(T, B, D)[:, 2 * p + g, dsl]
                out[core, 15 * p + r] = arr.reshape(-1)
            out[core, 15 * p + 12:15 * p + 15] = np.float16(1.0)
    return out


def kernel(x, W1, b1, W2, b2, threshold, T: int = T, **_unused):
    x = np.asarray(x, np.float32)
    W1 = np.asarray(W1, np.float32)
    b1 = np.asarray(b1, np.float32)
    W2 = np.asarray(W2, np.float32)
    b2 = np.atleast_1d(np.asarray(b2, np.float32))
    thr = float(np.asarray(threshold))
    assert x.shape == (128, 4096), x.shape

    key = (STRATS, tuple(sorted(INJ_DVE_PAIRS)), W2_SPLIT, WAVE)
    if key not in _prog_cache:
        _prog_cache[key] = _build_program()
    nc = _prog_cache[key]

    wh, wy, idm, b2c = _prep_inputs(x, W1, b1, W2, b2, thr)
    xstage = _build_xstage(x)

    in_maps = []
    for core in range(NCORES):
        in_maps.append({
            "xstage": xstage[core],
            "wh": wh,
            "wy": wy,
            "idm": idm,
            "b2c": b2c,
        })
    global LAST_EXEC_NS
    kw = {}
    if TRACE:
        _ensure_ntff_hook()
        kw = dict(trace=True, trace_cores=[0])
    res = run_bass_kernel_spmd(nc, in_maps, core_ids=list(range(NCORES)), **kw)
    if TRACE:
        LAST_EXEC_NS = res.exec_time_ns
    out = np.concatenate([res.results[i]["y"] for i in range(NCORES)], axis=1)
    return out.astype(np.float32)


# revision 33
# speedup vs baseline: 1.0708x; 1.0261x over previous
"""Trainium2 Bass kernel for nn_NonLinearOp (integrate-and-fire scan).

Math per element x[t,b,d] (scalar v):
  h[n] = x*W1[n] + b1[n]            (n = 64 neurons)
  scan over t: v += h_t; spike = thr*(v>=thr); v -= spike
  y[t,b,d] = sum_n spike[t,b,d,n]*W2[n] + b2

Device mapping (8 cores, D sharded 4096 -> 512 per core):
  - partitions = 64 neurons x 2 b-values (128); free dim = 512 d-cols.
  - TensorE accumulates v (normalized by 1/thr) in PSUM via K=15 fp16
    triple-split matmuls (products fp32-exact), one per T-step per pair.
  - spike indicator:
      strat 'A': DVE tensor_scalar is_ge (exact, incl. v==thr)
      strat 'B': ScalarE Sign(v-1) -> s' in {-1,0,1}; affine terms folded
                 into the W1/b1 weight rows, W2 scaling, and b2.
  - soft reset: DVE subtract (A) / DVE fused stt or PE -0.5*Id matmul (B)
  - y: TensorE K=128 matmul with fp16 W2, M=32 slots (32-aligned, zero-pad
    cols) accumulated per pair in a 2-bank PSUM tile; evac + b2 on DVE.
  - software-pipelined wavefront: 4 pairs per wave step-interleaved, y
    matmuls of wave w-1 run inside wave w's scan slots.
"""

import numpy as np

import concourse.bass as bass
import concourse.bacc as bacc
import concourse.mybir as mybir
import concourse.tile as tile
from concourse.bass_utils import run_bass_kernel_spmd

F16 = mybir.dt.float16
F32 = mybir.dt.float32

T, B, D, N = 8, 16, 4096, 64
NCORES = 8
DLOC = D // NCORES          # 512
PAIRS = 8                   # b-blocks of 2 -> groups A/B = single b each
FREE = DLOC                 # 512 free elements per pair tile

# ---- tuning knobs ----
STRATS = ("B",) * 8             # per-pair: 'A' (DVE is_ge+sub) or 'B' (ACT Sign)
INJ_DVE_PAIRS = frozenset(range(8))  # B-pairs injecting on DVE
W2_SPLIT = False                # hi/lo split of W2 contraction
WAVE = 6                        # pairs per pipeline wave
EVAC_ACT_PAIRS = frozenset({0, 4})  # pairs whose evac runs on ScalarE

_prog_cache = {}
TRACE = False          # set by test.py; harness leaves it False
LAST_EXEC_NS = None    # filled from the NTFF profile when TRACE


def _ensure_ntff_hook():
    """The container image's antenv lacks axon_hooks; synthesize it so
    run_bass_kernel_spmd(trace=True) can capture NTFF profiles."""
    import sys as _sys
    if "antenv.axon_hooks" in _sys.modules:
        return
    import contextlib
    import ctypes
    import types

    so_path = "/opt/axon/libaxon_pjrt.so"
    try:
        lib = ctypes.CDLL(so_path)
    except OSError:
        return
    if not hasattr(lib, "axon_start_nrt_profile"):
        return
    lib.axon_start_nrt_profile.argtypes = [ctypes.POINTER(ctypes.c_int64),
                                           ctypes.c_size_t]
    lib.axon_start_nrt_profile.restype = ctypes.c_int64
    lib.axon_stop_nrt_profile.argtypes = [ctypes.c_char_p]
    lib.axon_stop_nrt_profile.restype = ctypes.c_int64

    @contextlib.contextmanager
    def _hook(output_dir, device_ids):
        import jax
        jax.devices()
        if device_ids:
            ids = (ctypes.c_int64 * len(device_ids))(*device_ids)
            rc = lib.axon_start_nrt_profile(ids, len(device_ids))
        else:
            rc = lib.axon_start_nrt_profile(None, 0)
        if rc != 0:
            raise RuntimeError(f"axon_start_nrt_profile rc={rc}")
        try:
            yield
        finally:
            n = lib.axon_stop_nrt_profile(str(output_dir).encode())
            print(f"ntff profile: {n} file(s) written to {output_dir}")

    mod = types.ModuleType("antenv.axon_hooks")
    mod.get_axon_ntff_profile_hook = lambda: _hook
    mod.set_axon_ntff_profile_hook = lambda h: None
    _sys.modules["antenv.axon_hooks"] = mod


def _dedup_ldweights(nc):
    """Drop PE-stream-consecutive InstLdweights with identical weight APs
    (PE weight regs persist across matmuls); only sem-free dups removed."""
    removed = 0
    for blk in nc.m.functions[0].blocks:
        keep = []
        last_sig = None
        for ins in blk.instructions:
            if getattr(ins, "engine", None) == mybir.EngineType.PE:
                nm = type(ins).__name__
                if nm == "InstLdweights":
                    sig = repr(ins.ins[0])
                    si = ins.sync_info
                    clean = si is None or (not si.on_wait and not si.on_update)
                    if sig == last_sig and clean:
                        removed += 1
                        continue
                    last_sig = sig
                elif nm not in ("InstMatmult", "InstNop", "InstEventSemaphore"):
                    last_sig = None
            keep.append(ins)
        blk.instructions[:] = keep
    return removed


def _split3(v):
    """fp32 -> 3x fp16 (hi, mid, lo) with hi+mid+lo ~ v to ~2^-33."""
    v = v.astype(np.float32)
    hi = v.astype(np.float16)
    r = v - hi.astype(np.float32)
    mid = r.astype(np.float16)
    lo = (r - mid.astype(np.float32)).astype(np.float16)
    return hi, mid, lo


def _build_program():
    """One SPMD program; all weights/data arrive as ExternalInputs."""
    nc = bacc.Bacc(None, target_bir_lowering=False)
    xstage = nc.dram_tensor("xstage", [PAIRS * 15, T * FREE], F16,
                            kind="ExternalInput")
    wh = nc.dram_tensor("wh", [30, 128], F16, kind="ExternalInput")
    wy = nc.dram_tensor("wy", [128, 128], F16, kind="ExternalInput")
    idm = nc.dram_tensor("idm", [128, 128], F16, kind="ExternalInput")
    b2c = nc.dram_tensor("b2c", [128, 4], F32, kind="ExternalInput")
    yout = nc.dram_tensor("y", [T * B, DLOC], F32, kind="ExternalOutput")

    with tile.TileContext(nc) as tc:
        with (
            tc.tile_pool(name="const", bufs=1) as constp,
            tc.tile_pool(name="stage", bufs=2 * WAVE) as stagep,
            tc.tile_pool(name="spool", bufs=48) as spool,
            tc.tile_pool(name="ysb", bufs=2) as ysbp,
            tc.tile_pool(name="vps", bufs=WAVE, space="PSUM") as vpool,
            tc.tile_pool(name="yps", bufs=1, space="PSUM") as ypool,
        ):
            wh_t = constp.tile([15, 128], F16)
            nc.sync.dma_start(out=wh_t[:], in_=wh[0:15, :])
            wh_r = constp.tile([15, 128], F16)
            nc.gpsimd.dma_start(out=wh_r[:], in_=wh[15:30, :])
            b2_t = constp.tile([128, 4], F32)
            nc.scalar.dma_start(out=b2_t[:], in_=b2c[:])
            wy_t = constp.tile([128, 128], F16)
            nc.scalar.dma_start(out=wy_t[:], in_=wy[:])
            id_t = constp.tile([128, 128], F16)
            nc.scalar.dma_start(out=id_t[:], in_=idm[:])

            n_waves = PAIRS // WAVE
            s_tiles = {}
            v_tiles = {}
            stage_tiles = {}
            ysl_tiles = {}

            def stage_load(p):
                stage = stagep.tile([15, T, FREE], F16, name=f"stage{p}",
                                    tag="stage")
                stage_tiles[p] = stage
                (nc.sync if p % 2 == 0 else nc.gpsimd).dma_start(
                    out=stage[:, :, :],
                    in_=xstage[15 * p:15 * (p + 1), :].rearrange(
                        "r (t f) -> r t f", t=T),
                )

            def emit_y_phase(p, thalf):
                """y matmuls for pair p, t in [4*thalf, 4*thalf+4)."""
                strat = STRATS[p % len(STRATS)]
                wyo = 64 if strat == "B" else 0
                if thalf == 0:
                    ysl_tiles[p] = ypool.tile([128, 2 * FREE], F32,
                                              name=f"ysl{p}", tag="ysl")
                ysl = ysl_tiles[p]
                for t in range(4 * thalf, 4 * thalf + 4):
                    s = s_tiles.pop((p, t))
                    prow = 32 * (t % 4)
                    cbase = (t // 4) * FREE
                    outsl = ysl[prow:prow + 32, cbase:cbase + FREE]
                    tp = (0, prow)
                    if W2_SPLIT:
                        nc.tensor.matmul(outsl, lhsT=wy_t[:, wyo:wyo + 32],
                                         rhs=s[:, :], start=True, stop=False,
                                         tile_position=tp)
                        nc.tensor.matmul(outsl, lhsT=wy_t[:, wyo + 32:wyo + 64],
                                         rhs=s[:, :], start=False, stop=True,
                                         tile_position=tp)
                    else:
                        nc.tensor.matmul(outsl, lhsT=wy_t[:, wyo:wyo + 32],
                                         rhs=s[:, :], start=True, stop=True,
                                         tile_position=tp)
                if thalf == 1:
                    emit_evac(p, ysl)

            def emit_evac(p, ysl):
                strat = STRATS[p % len(STRATS)]
                y_sb = ysbp.tile([128, 2 * FREE], F32, name=f"ysb{p}",
                                 tag="ysb")
                bcol = 0 if strat == "A" else 1
                if p in EVAC_ACT_PAIRS:
                    nc.scalar.activation(
                        out=y_sb[:], in_=ysl[:],
                        func=mybir.ActivationFunctionType.Identity,
                        bias=b2_t[:, bcol:bcol + 1], scale=1.0,
                    )
                else:
                    nc.vector.tensor_scalar(
                        out=y_sb[:], in0=ysl[:], scalar1=b2_t[:, bcol:bcol + 1],
                        scalar2=None, op0=mybir.AluOpType.add,
                    )
                ypext = y_sb.ap[0][0]
                for k in range(4):
                    # t = k and t = k+4 share src partitions {32k, 32k+1}
                    src_ap = bass.AP(
                        tensor=y_sb.tensor,
                        offset=y_sb.offset + 32 * k * ypext,
                        ap=[[ypext, 2], [FREE, 2], [1, FREE]],
                    )
                    dst_ap = bass.AP(
                        tensor=yout,
                        offset=(k * B + 2 * p) * DLOC,
                        ap=[[DLOC, 2], [4 * B * DLOC, 2], [1, DLOC]],
                    )
                    eng = nc.gpsimd if k % 2 == 0 else nc.sync
                    eng.dma_start(out=dst_ap, in_=src_ap)

            for p in range(min(WAVE, PAIRS)):
                stage_load(p)

            yq = []           # ready y-units (pair, thalf)
            loaded = min(WAVE, PAIRS)
            waves = []
            q = 0
            while q < PAIRS:
                waves.append(list(range(q, min(q + WAVE, PAIRS))))
                q += WAVE
            for wave in waves:
                for t in range(T):
                    # drain ready y work into this slot (up to 2 units)
                    for _ in range(2):
                        if yq:
                            pp, hh = yq.pop(0)
                            emit_y_phase(pp, hh)
                    # --- h accumulate ---
                    for p in wave:
                        strat = STRATS[p % len(STRATS)]
                        if t == 0:
                            v_tiles[p] = vpool.tile([128, FREE], F32,
                                                    name=f"v{p}", tag="v")
                        whv = wh_t[:] if (strat == "A" or t == 0) else wh_r[:]
                        nc.tensor.matmul(
                            v_tiles[p][:, :], lhsT=whv,
                            rhs=stage_tiles[p][:, t, :],
                            start=(t == 0), stop=True, skip_group_check=True,
                        )
                    # --- spike indicator ---
                    for p in wave:
                        strat = STRATS[p % len(STRATS)]
                        s = spool.tile([128, FREE], F16, name=f"s{p}_{t}",
                                       tag="s")
                        s_tiles[(p, t)] = s
                        v = v_tiles[p]
                        if strat == "A":
                            nc.vector.tensor_scalar(
                                out=s[:], in0=v[:], scalar1=1.0, scalar2=None,
                                op0=mybir.AluOpType.is_ge,
                            )
                        else:
                            nc.scalar.activation(
                                out=s[:], in_=v[:],
                                func=mybir.ActivationFunctionType.Sign,
                                bias=b2_t[:, 2:3], scale=1.0,
                            )
                        if t == 3:
                            yq.append((p, 0))
                        elif t == T - 1:
                            yq.append((p, 1))
                    # --- reset inject ---
                    if t < T - 1:
                        for p in wave:
                            strat = STRATS[p % len(STRATS)]
                            s, v = s_tiles[(p, t)], v_tiles[p]
                            if strat == "A":
                                nc.vector.tensor_tensor(
                                    out=v[:], in0=v[:], in1=s[:],
                                    op=mybir.AluOpType.subtract,
                                )
                            elif p in INJ_DVE_PAIRS:
                                nc.vector.scalar_tensor_tensor(
                                    out=v[:], in0=s[:], scalar=-0.5, in1=v[:],
                                    op0=mybir.AluOpType.mult,
                                    op1=mybir.AluOpType.add,
                                )
                            else:
                                nc.tensor.matmul(
                                    v[:, :], lhsT=id_t[:, :], rhs=s[:, :],
                                    start=False, stop=True,
                                    skip_group_check=True,
                                )
                    if loaded < PAIRS:
                        stage_load(loaded)
                        loaded += 1
            # tail: drain remaining y work
            for pp, hh in yq:
                emit_y_phase(pp, hh)
    nc.compile()
    return nc


def _prep_inputs(x, W1, b1, W2, b2, thr):
    a = W1[:, 0].astype(np.float32) / thr
    c = b1.astype(np.float32) / thr
    ah, am, al = _split3(a)
    w2p = thr * W2[0].astype(np.float32)

    wh = np.zeros((30, 128), np.float16)
    for base, cvals in ((0, c), (15, c - 0.5)):
        ch, cm, cl = _split3(cvals)
        for g in range(2):
            cols = slice(64 * g, 64 * (g + 1))
            rows = base + 6 * g
            wh[rows + 0, cols] = ah   # pairs x_hi
            wh[rows + 1, cols] = am   # pairs x_hi
            wh[rows + 2, cols] = al   # pairs x_hi
            wh[rows + 3, cols] = ah   # pairs x_mid
            wh[rows + 4, cols] = am   # pairs x_mid
            wh[rows + 5, cols] = ah   # pairs x_lo
        wh[base + 12, :64] = wh[base + 12, 64:] = ch
        wh[base + 13, :64] = wh[base + 13, 64:] = cm
        wh[base + 14, :64] = wh[base + 14, 64:] = cl

    wy = np.zeros((128, 128), np.float16)
    for varbase, scale in ((0, 1.0), (64, 0.5)):
        vv = (scale * w2p).astype(np.float32)
        vh = vv.astype(np.float16)
        vl = (vv - vh.astype(np.float32)).astype(np.float16)
        for g in range(2):
            rows = slice(64 * g, 64 * (g + 1))
            wy[rows, varbase + g] = vh          # hi block cols 0,1
            wy[rows, varbase + 32 + g] = vl     # lo block cols 32,33
    idm = (-0.5 * np.eye(128, dtype=np.float32)).astype(np.float16)
    b2c = np.zeros((128, 4), np.float32)
    b2c[:, 0] = np.float32(b2[0])
    b2c[:, 1] = np.float32(b2[0] + 0.5 * w2p.sum(dtype=np.float32))
    b2c[:, 2] = np.float32(-1.0)
    return wh, wy, idm, b2c


def _build_xstage(x):
    """Per-core staged x layout: [NCORES][PAIRS*15, T*FREE] fp16.

    rows 0-2: x_hi grpA (x3), 3-4: x_mid A, 5: x_lo A, 6-11: same grpB,
    12-14: ones.  free = t*FREE + d,  b = 2p + g.
    """
    xh, xm_, xl_ = _split3(x)                       # [T*B, D]
    out = np.empty((NCORES, PAIRS * 15, T * FREE), np.float16)
    rowmap = [(0, 0), (0, 0), (0, 0), (1, 0), (1, 0), (2, 0),
              (0, 1), (0, 1), (0, 1), (1, 1), (1, 1), (2, 1)]
    parts = (xh, xm_, xl_)
    for core in range(NCORES):
        dsl = slice(core * DLOC, (core + 1) * DLOC)
        for p in range(PAIRS):
            for r, (pi, g) in enumerate(rowmap):
                arr = parts[pi].reshape(T, B, D)[:, 2 * p + g, dsl]
                out[core, 15 * p + r] = arr.reshape(-1)
            out[core, 15 * p + 12:15 * p + 15] = np.float16(1.0)
    return out


def kernel(x, W1, b1, W2, b2, threshold, T: int = T, **_unused):
    x = np.asarray(x, np.float32)
    W1 = np.asarray(W1, np.float32)
    b1 = np.asarray(b1, np.float32)
    W2 = np.asarray(W2, np.float32)
    b2 = np.atleast_1d(np.asarray(b2, np.float32))
    thr = float(np.asarray(threshold))
    assert x.shape == (128, 4096), x.shape

    key = (STRATS, tuple(sorted(INJ_DVE_PAIRS)), W2_SPLIT, WAVE)
    if key not in _prog_cache:
        _prog_cache[key] = _build_program()
    nc = _prog_cache[key]

    wh, wy, idm, b2c = _prep_inputs(x, W1, b1, W2, b2, thr)
    xstage = _build_xstage(x)

    in_maps = []
    for core in range(NCORES):
        in_maps.append({
            "xstage": xstage[core],
            "wh": wh,
            "wy": wy,
            "idm": idm,
            "b2c": b2c,
        })
    global LAST_EXEC_NS
    kw = {}
    if TRACE:
        _ensure_ntff_hook()
        kw = dict(trace=True, trace_cores=[0])
    res = run_bass_kernel_spmd(nc, in_maps, core_ids=list(range(NCORES)), **kw)
    if TRACE:
        LAST_EXEC_NS = res.exec_time_ns
    out = np.concatenate([res.results[i]["y"] for i in range(NCORES)], axis=1)
    return out.astype(np.float32)


# revision 34
# speedup vs baseline: 1.0729x; 1.0019x over previous
"""Trainium2 Bass kernel for nn_NonLinearOp (integrate-and-fire scan).

Math per element x[t,b,d] (scalar v):
  h[n] = x*W1[n] + b1[n]            (n = 64 neurons)
  scan over t: v += h_t; spike = thr*(v>=thr); v -= spike
  y[t,b,d] = sum_n spike[t,b,d,n]*W2[n] + b2

Device mapping (8 cores, D sharded 4096 -> 512 per core):
  - partitions = 64 neurons x 2 b-values (128); free dim = 512 d-cols.
  - TensorE accumulates v (normalized by 1/thr) in PSUM via K=15 fp16
    triple-split matmuls (products fp32-exact), one per T-step per pair.
  - spike indicator:
      strat 'A': DVE tensor_scalar is_ge (exact, incl. v==thr)
      strat 'B': ScalarE Sign(v-1) -> s' in {-1,0,1}; affine terms folded
                 into the W1/b1 weight rows, W2 scaling, and b2.
  - soft reset: DVE subtract (A) / DVE fused stt or PE -0.5*Id matmul (B)
  - y: TensorE K=128 matmul with fp16 W2, M=32 slots (32-aligned, zero-pad
    cols) accumulated per pair in a 2-bank PSUM tile; evac + b2 on DVE.
  - software-pipelined wavefront: 4 pairs per wave step-interleaved, y
    matmuls of wave w-1 run inside wave w's scan slots.
"""

import numpy as np

import concourse.bass as bass
import concourse.bacc as bacc
import concourse.mybir as mybir
import concourse.tile as tile
from concourse.bass_utils import run_bass_kernel_spmd

F16 = mybir.dt.float16
F32 = mybir.dt.float32

T, B, D, N = 8, 16, 4096, 64
NCORES = 8
DLOC = D // NCORES          # 512
PAIRS = 8                   # b-blocks of 2 -> groups A/B = single b each
FREE = DLOC                 # 512 free elements per pair tile

# ---- tuning knobs ----
STRATS = ("B",) * 8             # per-pair: 'A' (DVE is_ge+sub) or 'B' (ACT Sign)
INJ_DVE_PAIRS = frozenset(range(8))  # B-pairs injecting on DVE
W2_SPLIT = False                # hi/lo split of W2 contraction
WAVE = 5                        # pairs per pipeline wave
EVAC_ACT_PAIRS = frozenset({0, 4})  # pairs whose evac runs on ScalarE

_prog_cache = {}
TRACE = False          # set by test.py; harness leaves it False
LAST_EXEC_NS = None    # filled from the NTFF profile when TRACE


def _ensure_ntff_hook():
    """The container image's antenv lacks axon_hooks; synthesize it so
    run_bass_kernel_spmd(trace=True) can capture NTFF profiles."""
    import sys as _sys
    if "antenv.axon_hooks" in _sys.modules:
        return
    import contextlib
    import ctypes
    import types

    so_path = "/opt/axon/libaxon_pjrt.so"
    try:
        lib = ctypes.CDLL(so_path)
    except OSError:
        return
    if not hasattr(lib, "axon_start_nrt_profile"):
        return
    lib.axon_start_nrt_profile.argtypes = [ctypes.POINTER(ctypes.c_int64),
                                           ctypes.c_size_t]
    lib.axon_start_nrt_profile.restype = ctypes.c_int64
    lib.axon_stop_nrt_profile.argtypes = [ctypes.c_char_p]
    lib.axon_stop_nrt_profile.restype = ctypes.c_int64

    @contextlib.contextmanager
    def _hook(output_dir, device_ids):
        import jax
        jax.devices()
        if device_ids:
            ids = (ctypes.c_int64 * len(device_ids))(*device_ids)
            rc = lib.axon_start_nrt_profile(ids, len(device_ids))
        else:
            rc = lib.axon_start_nrt_profile(None, 0)
        if rc != 0:
            raise RuntimeError(f"axon_start_nrt_profile rc={rc}")
        try:
            yield
        finally:
            n = lib.axon_stop_nrt_profile(str(output_dir).encode())
            print(f"ntff profile: {n} file(s) written to {output_dir}")

    mod = types.ModuleType("antenv.axon_hooks")
    mod.get_axon_ntff_profile_hook = lambda: _hook
    mod.set_axon_ntff_profile_hook = lambda h: None
    _sys.modules["antenv.axon_hooks"] = mod


def _dedup_ldweights(nc):
    """Drop PE-stream-consecutive InstLdweights with identical weight APs
    (PE weight regs persist across matmuls); only sem-free dups removed."""
    removed = 0
    for blk in nc.m.functions[0].blocks:
        keep = []
        last_sig = None
        for ins in blk.instructions:
            if getattr(ins, "engine", None) == mybir.EngineType.PE:
                nm = type(ins).__name__
                if nm == "InstLdweights":
                    sig = repr(ins.ins[0])
                    si = ins.sync_info
                    clean = si is None or (not si.on_wait and not si.on_update)
                    if sig == last_sig and clean:
                        removed += 1
                        continue
                    last_sig = sig
                elif nm not in ("InstMatmult", "InstNop", "InstEventSemaphore"):
                    last_sig = None
            keep.append(ins)
        blk.instructions[:] = keep
    return removed


def _split3(v):
    """fp32 -> 3x fp16 (hi, mid, lo) with hi+mid+lo ~ v to ~2^-33."""
    v = v.astype(np.float32)
    hi = v.astype(np.float16)
    r = v - hi.astype(np.float32)
    mid = r.astype(np.float16)
    lo = (r - mid.astype(np.float32)).astype(np.float16)
    return hi, mid, lo


def _build_program():
    """One SPMD program; all weights/data arrive as ExternalInputs."""
    nc = bacc.Bacc(None, target_bir_lowering=False)
    xstage = nc.dram_tensor("xstage", [PAIRS * 15, T * FREE], F16,
                            kind="ExternalInput")
    wh = nc.dram_tensor("wh", [30, 128], F16, kind="ExternalInput")
    wy = nc.dram_tensor("wy", [128, 128], F16, kind="ExternalInput")
    idm = nc.dram_tensor("idm", [128, 128], F16, kind="ExternalInput")
    b2c = nc.dram_tensor("b2c", [128, 4], F32, kind="ExternalInput")
    yout = nc.dram_tensor("y", [T * B, DLOC], F32, kind="ExternalOutput")

    with tile.TileContext(nc) as tc:
        with (
            tc.tile_pool(name="const", bufs=1) as constp,
            tc.tile_pool(name="stage", bufs=2 * WAVE) as stagep,
            tc.tile_pool(name="spool", bufs=48) as spool,
            tc.tile_pool(name="ysb", bufs=2) as ysbp,
            tc.tile_pool(name="vps", bufs=WAVE, space="PSUM") as vpool,
            tc.tile_pool(name="yps", bufs=1, space="PSUM") as ypool,
        ):
            wh_t = constp.tile([15, 128], F16)
            nc.sync.dma_start(out=wh_t[:], in_=wh[0:15, :])
            wh_r = constp.tile([15, 128], F16)
            nc.gpsimd.dma_start(out=wh_r[:], in_=wh[15:30, :])
            b2_t = constp.tile([128, 4], F32)
            nc.scalar.dma_start(out=b2_t[:], in_=b2c[:])
            wy_t = constp.tile([128, 128], F16)
            nc.scalar.dma_start(out=wy_t[:], in_=wy[:])
            id_t = constp.tile([128, 128], F16)
            nc.scalar.dma_start(out=id_t[:], in_=idm[:])

            n_waves = PAIRS // WAVE
            s_tiles = {}
            v_tiles = {}
            stage_tiles = {}
            ysl_tiles = {}

            def stage_load(p):
                stage = stagep.tile([15, T, FREE], F16, name=f"stage{p}",
                                    tag="stage")
                stage_tiles[p] = stage
                (nc.sync if p % 2 == 0 else nc.gpsimd).dma_start(
                    out=stage[:, :, :],
                    in_=xstage[15 * p:15 * (p + 1), :].rearrange(
                        "r (t f) -> r t f", t=T),
                )

            def emit_y_phase(p, thalf):
                """y matmuls for pair p, t in [4*thalf, 4*thalf+4)."""
                strat = STRATS[p % len(STRATS)]
                wyo = 64 if strat == "B" else 0
                if thalf == 0:
                    ysl_tiles[p] = ypool.tile([128, 2 * FREE], F32,
                                              name=f"ysl{p}", tag="ysl")
                ysl = ysl_tiles[p]
                for t in range(4 * thalf, 4 * thalf + 4):
                    s = s_tiles.pop((p, t))
                    prow = 32 * (t % 4)
                    cbase = (t // 4) * FREE
                    outsl = ysl[prow:prow + 32, cbase:cbase + FREE]
                    tp = (0, prow)
                    if W2_SPLIT:
                        nc.tensor.matmul(outsl, lhsT=wy_t[:, wyo:wyo + 32],
                                         rhs=s[:, :], start=True, stop=False,
                                         tile_position=tp)
                        nc.tensor.matmul(outsl, lhsT=wy_t[:, wyo + 32:wyo + 64],
                                         rhs=s[:, :], start=False, stop=True,
                                         tile_position=tp)
                    else:
                        nc.tensor.matmul(outsl, lhsT=wy_t[:, wyo:wyo + 32],
                                         rhs=s[:, :], start=True, stop=True,
                                         tile_position=tp)
                if thalf == 1:
                    emit_evac(p, ysl)

            def emit_evac(p, ysl):
                strat = STRATS[p % len(STRATS)]
                y_sb = ysbp.tile([128, 2 * FREE], F32, name=f"ysb{p}",
                                 tag="ysb")
                bcol = 0 if strat == "A" else 1
                if p in EVAC_ACT_PAIRS:
                    nc.scalar.activation(
                        out=y_sb[:], in_=ysl[:],
                        func=mybir.ActivationFunctionType.Identity,
                        bias=b2_t[:, bcol:bcol + 1], scale=1.0,
                    )
                else:
                    nc.vector.tensor_scalar(
                        out=y_sb[:], in0=ysl[:], scalar1=b2_t[:, bcol:bcol + 1],
                        scalar2=None, op0=mybir.AluOpType.add,
                    )
                ypext = y_sb.ap[0][0]
                for k in range(4):
                    # t = k and t = k+4 share src partitions {32k, 32k+1}
                    src_ap = bass.AP(
                        tensor=y_sb.tensor,
                        offset=y_sb.offset + 32 * k * ypext,
                        ap=[[ypext, 2], [FREE, 2], [1, FREE]],
                    )
                    dst_ap = bass.AP(
                        tensor=yout,
                        offset=(k * B + 2 * p) * DLOC,
                        ap=[[DLOC, 2], [4 * B * DLOC, 2], [1, DLOC]],
                    )
                    eng = nc.gpsimd if k % 2 == 0 else nc.sync
                    eng.dma_start(out=dst_ap, in_=src_ap)

            for p in range(min(WAVE, PAIRS)):
                stage_load(p)

            yq = []           # ready y-units (pair, thalf)
            loaded = min(WAVE, PAIRS)
            waves = []
            q = 0
            while q < PAIRS:
                waves.append(list(range(q, min(q + WAVE, PAIRS))))
                q += WAVE
            for wave in waves:
                for t in range(T):
                    # drain ready y work into this slot (up to 2 units)
                    for _ in range(2):
                        if yq:
                            pp, hh = yq.pop(0)
                            emit_y_phase(pp, hh)
                    # --- h accumulate ---
                    for p in wave:
                        strat = STRATS[p % len(STRATS)]
                        if t == 0:
                            v_tiles[p] = vpool.tile([128, FREE], F32,
                                                    name=f"v{p}", tag="v")
                        whv = wh_t[:] if (strat == "A" or t == 0) else wh_r[:]
                        nc.tensor.matmul(
                            v_tiles[p][:, :], lhsT=whv,
                            rhs=stage_tiles[p][:, t, :],
                            start=(t == 0), stop=True, skip_group_check=True,
                        )
                    # --- spike indicator ---
                    for p in wave:
                        strat = STRATS[p % len(STRATS)]
                        s = spool.tile([128, FREE], F16, name=f"s{p}_{t}",
                                       tag="s")
                        s_tiles[(p, t)] = s
                        v = v_tiles[p]
                        if strat == "A":
                            nc.vector.tensor_scalar(
                                out=s[:], in0=v[:], scalar1=1.0, scalar2=None,
                                op0=mybir.AluOpType.is_ge,
                            )
                        else:
                            nc.scalar.activation(
                                out=s[:], in_=v[:],
                                func=mybir.ActivationFunctionType.Sign,
                                bias=b2_t[:, 2:3], scale=1.0,
                            )
                        if t == 3:
                            yq.append((p, 0))
                        elif t == T - 1:
                            yq.append((p, 1))
                    # --- reset inject ---
                    if t < T - 1:
                        for p in wave:
                            strat = STRATS[p % len(STRATS)]
                            s, v = s_tiles[(p, t)], v_tiles[p]
                            if strat == "A":
                                nc.vector.tensor_tensor(
                                    out=v[:], in0=v[:], in1=s[:],
                                    op=mybir.AluOpType.subtract,
                                )
                            elif p in INJ_DVE_PAIRS:
                                nc.vector.scalar_tensor_tensor(
                                    out=v[:], in0=s[:], scalar=-0.5, in1=v[:],
                                    op0=mybir.AluOpType.mult,
                                    op1=mybir.AluOpType.add,
                                )
                            else:
                                nc.tensor.matmul(
                                    v[:, :], lhsT=id_t[:, :], rhs=s[:, :],
                                    start=False, stop=True,
                                    skip_group_check=True,
                                )
                    if loaded < PAIRS:
                        stage_load(loaded)
                        loaded += 1
            # tail: drain remaining y work
            for pp, hh in yq:
                emit_y_phase(pp, hh)
    nc.compile()
    return nc


def _prep_inputs(x, W1, b1, W2, b2, thr):
    a = W1[:, 0].astype(np.float32) / thr
    c = b1.astype(np.float32) / thr
    ah, am, al = _split3(a)
    w2p = thr * W2[0].astype(np.float32)

    wh = np.zeros((30, 128), np.float16)
    for base, cvals in ((0, c), (15, c - 0.5)):
        ch, cm, cl = _split3(cvals)
        for g in range(2):
            cols = slice(64 * g, 64 * (g + 1))
            rows = base + 6 * g
            wh[rows + 0, cols] = ah   # pairs x_hi
            wh[rows + 1, cols] = am   # pairs x_hi
            wh[rows + 2, cols] = al   # pairs x_hi
            wh[rows + 3, cols] = ah   # pairs x_mid
            wh[rows + 4, cols] = am   # pairs x_mid
            wh[rows + 5, cols] = ah   # pairs x_lo
        wh[base + 12, :64] = wh[base + 12, 64:] = ch
        wh[base + 13, :64] = wh[base + 13, 64:] = cm
        wh[base + 14, :64] = wh[base + 14, 64:] = cl

    wy = np.zeros((128, 128), np.float16)
    for varbase, scale in ((0, 1.0), (64, 0.5)):
        vv = (scale * w2p).astype(np.float32)
        vh = vv.astype(np.float16)
        vl = (vv - vh.astype(np.float32)).astype(np.float16)
        for g in range(2):
            rows = slice(64 * g, 64 * (g + 1))
            wy[rows, varbase + g] = vh          # hi block cols 0,1
            wy[rows, varbase + 32 + g] = vl     # lo block cols 32,33
    idm = (-0.5 * np.eye(128, dtype=np.float32)).astype(np.float16)
    b2c = np.zeros((128, 4), np.float32)
    b2c[:, 0] = np.float32(b2[0])
    b2c[:, 1] = np.float32(b2[0] + 0.5 * w2p.sum(dtype=np.float32))
    b2c[:, 2] = np.float32(-1.0)
    return wh, wy, idm, b2c


def _build_xstage(x):
    """Per-core staged x layout: [NCORES][PAIRS*15, T*FREE] fp16.

    rows 0-2: x_hi grpA (x3), 3-4: x_mid A, 5: x_lo A, 6-11: same grpB,
    12-14: ones.  free = t*FREE + d,  b = 2p + g.
    """
    xh, xm_, xl_ = _split3(x)                       # [T*B, D]
    out = np.empty((NCORES, PAIRS * 15, T * FREE), np.float16)
    rowmap = [(0, 0), (0, 0), (0, 0), (1, 0), (1, 0), (2, 0),
              (0, 1), (0, 1), (0, 1), (1, 1), (1, 1), (2, 1)]
    parts = (xh, xm_, xl_)
    for core in range(NCORES):
        dsl = slice(core * DLOC, (core + 1) * DLOC)
        for p in range(PAIRS):
            for r, (pi, g) in enumerate(rowmap):
                arr = parts[pi].reshape(T, B, D)[:, 2 * p + g, dsl]
                out[core, 15 * p + r] = arr.reshape(-1)
            out[core, 15 * p + 12:15 * p + 15] = np.float16(1.0)
    return out


def kernel(x, W1, b1, W2, b2, threshold, T: int = T, **_unused):
    x = np.asarray(x, np.float32)
    W1 = np.asarray(W1, np.float32)
    b1 = np.asarray(b1, np.float32)
    W2 = np.asarray(W2, np.float32)
    b2 = np.atleast_1d(np.asarray(b2, np.float32))
    thr = float(np.asarray(threshold))
    assert x.shape == (128, 4096), x.shape

    key = (STRATS, tuple(sorted(INJ_DVE_PAIRS)), W2_SPLIT, WAVE)
    if key not in _prog_cache:
        _prog_cache[key] = _build_program()
    nc = _prog_cache[key]

    wh, wy, idm, b2c = _prep_inputs(x, W1, b1, W2, b2, thr)
    xstage = _build_xstage(x)

    in_maps = []
    for core in range(NCORES):
        in_maps.append({
            "xstage": xstage[core],
            "wh": wh,
            "wy": wy,
            "idm": idm,
            "b2c": b2c,
        })
    global LAST_EXEC_NS
    kw = {}
    if TRACE:
        _ensure_ntff_hook()
        kw = dict(trace=True, trace_cores=[0])
    res = run_bass_kernel_spmd(nc, in_maps, core_ids=list(range(NCORES)), **kw)
    if TRACE:
        LAST_EXEC_NS = res.exec_time_ns
    out = np.concatenate([res.results[i]["y"] for i in range(NCORES)], axis=1)
    return out.astype(np.float32)


# revision 36
# speedup vs baseline: 1.0751x; 1.0021x over previous
"""Trainium2 Bass kernel for nn_NonLinearOp (integrate-and-fire scan).

Math per element x[t,b,d] (scalar v):
  h[n] = x*W1[n] + b1[n]            (n = 64 neurons)
  scan over t: v += h_t; spike = thr*(v>=thr); v -= spike
  y[t,b,d] = sum_n spike[t,b,d,n]*W2[n] + b2

Device mapping (8 cores, D sharded 4096 -> 512 per core):
  - partitions = 64 neurons x 2 b-values (128); free dim = 512 d-cols.
  - TensorE accumulates v (normalized by 1/thr) in PSUM via K=15 fp16
    triple-split matmuls (products fp32-exact), one per T-step per pair.
  - spike indicator:
      strat 'A': DVE tensor_scalar is_ge (exact, incl. v==thr)
      strat 'B': ScalarE Sign(v-1) -> s' in {-1,0,1}; affine terms folded
                 into the W1/b1 weight rows, W2 scaling, and b2.
  - soft reset: DVE subtract (A) / DVE fused stt or PE -0.5*Id matmul (B)
  - y: TensorE K=128 matmul with fp16 W2, M=32 slots (32-aligned, zero-pad
    cols) accumulated per pair in a 2-bank PSUM tile; evac + b2 on DVE.
  - software-pipelined wavefront: 4 pairs per wave step-interleaved, y
    matmuls of wave w-1 run inside wave w's scan slots.
"""

import numpy as np

import concourse.bass as bass
import concourse.bacc as bacc
import concourse.mybir as mybir
import concourse.tile as tile
from concourse.bass_utils import run_bass_kernel_spmd

F16 = mybir.dt.float16
F32 = mybir.dt.float32

T, B, D, N = 8, 16, 4096, 64
NCORES = 8
DLOC = D // NCORES          # 512
PAIRS = 8                   # b-blocks of 2 -> groups A/B = single b each
FREE = DLOC                 # 512 free elements per pair tile

# ---- tuning knobs ----
STRATS = ("B",) * 8             # per-pair: 'A' (DVE is_ge+sub) or 'B' (ACT Sign)
INJ_DVE_PAIRS = frozenset(range(8))  # B-pairs injecting on DVE
W2_SPLIT = False                # hi/lo split of W2 contraction
WAVE = 5                        # pairs per pipeline wave
EVAC_ACT_PAIRS = frozenset({0, 4})  # pairs whose evac runs on ScalarE

_prog_cache = {}
TRACE = False          # set by test.py; harness leaves it False
LAST_EXEC_NS = None    # filled from the NTFF profile when TRACE


def _ensure_ntff_hook():
    """The container image's antenv lacks axon_hooks; synthesize it so
    run_bass_kernel_spmd(trace=True) can capture NTFF profiles."""
    import sys as _sys
    if "antenv.axon_hooks" in _sys.modules:
        return
    import contextlib
    import ctypes
    import types

    so_path = "/opt/axon/libaxon_pjrt.so"
    try:
        lib = ctypes.CDLL(so_path)
    except OSError:
        return
    if not hasattr(lib, "axon_start_nrt_profile"):
        return
    lib.axon_start_nrt_profile.argtypes = [ctypes.POINTER(ctypes.c_int64),
                                           ctypes.c_size_t]
    lib.axon_start_nrt_profile.restype = ctypes.c_int64
    lib.axon_stop_nrt_profile.argtypes = [ctypes.c_char_p]
    lib.axon_stop_nrt_profile.restype = ctypes.c_int64

    @contextlib.contextmanager
    def _hook(output_dir, device_ids):
        import jax
        jax.devices()
        if device_ids:
            ids = (ctypes.c_int64 * len(device_ids))(*device_ids)
            rc = lib.axon_start_nrt_profile(ids, len(device_ids))
        else:
            rc = lib.axon_start_nrt_profile(None, 0)
        if rc != 0:
            raise RuntimeError(f"axon_start_nrt_profile rc={rc}")
        try:
            yield
        finally:
            n = lib.axon_stop_nrt_profile(str(output_dir).encode())
            print(f"ntff profile: {n} file(s) written to {output_dir}")

    mod = types.ModuleType("antenv.axon_hooks")
    mod.get_axon_ntff_profile_hook = lambda: _hook
    mod.set_axon_ntff_profile_hook = lambda h: None
    _sys.modules["antenv.axon_hooks"] = mod


def _dedup_ldweights(nc):
    """Drop PE-stream-consecutive InstLdweights with identical weight APs
    (PE weight regs persist across matmuls); only sem-free dups removed."""
    removed = 0
    for blk in nc.m.functions[0].blocks:
        keep = []
        last_sig = None
        for ins in blk.instructions:
            if getattr(ins, "engine", None) == mybir.EngineType.PE:
                nm = type(ins).__name__
                if nm == "InstLdweights":
                    sig = repr(ins.ins[0])
                    si = ins.sync_info
                    clean = si is None or (not si.on_wait and not si.on_update)
                    if sig == last_sig and clean:
                        removed += 1
                        continue
                    last_sig = sig
                elif nm not in ("InstMatmult", "InstNop", "InstEventSemaphore"):
                    last_sig = None
            keep.append(ins)
        blk.instructions[:] = keep
    return removed


def _split3(v):
    """fp32 -> 3x fp16 (hi, mid, lo) with hi+mid+lo ~ v to ~2^-33."""
    v = v.astype(np.float32)
    hi = v.astype(np.float16)
    r = v - hi.astype(np.float32)
    mid = r.astype(np.float16)
    lo = (r - mid.astype(np.float32)).astype(np.float16)
    return hi, mid, lo


def _build_program():
    """One SPMD program; all weights/data arrive as ExternalInputs."""
    nc = bacc.Bacc(None, target_bir_lowering=False)
    xstage = nc.dram_tensor("xstage", [PAIRS * 15, T * FREE], F16,
                            kind="ExternalInput")
    wh = nc.dram_tensor("wh", [30, 128], F16, kind="ExternalInput")
    wy = nc.dram_tensor("wy", [128, 128], F16, kind="ExternalInput")
    idm = nc.dram_tensor("idm", [128, 128], F16, kind="ExternalInput")
    b2c = nc.dram_tensor("b2c", [128, 4], F32, kind="ExternalInput")
    yout = nc.dram_tensor("y", [T * B, DLOC], F32, kind="ExternalOutput")

    with tile.TileContext(nc) as tc:
        with (
            tc.tile_pool(name="const", bufs=1) as constp,
            tc.tile_pool(name="stage", bufs=2 * WAVE) as stagep,
            tc.tile_pool(name="spool", bufs=48) as spool,
            tc.tile_pool(name="ysb", bufs=2) as ysbp,
            tc.tile_pool(name="vps", bufs=WAVE + 1, space="PSUM") as vpool,
            tc.tile_pool(name="yps", bufs=1, space="PSUM") as ypool,
        ):
            wh_t = constp.tile([15, 128], F16)
            nc.sync.dma_start(out=wh_t[:], in_=wh[0:15, :])
            wh_r = constp.tile([15, 128], F16)
            nc.gpsimd.dma_start(out=wh_r[:], in_=wh[15:30, :])
            b2_t = constp.tile([128, 4], F32)
            nc.scalar.dma_start(out=b2_t[:], in_=b2c[:])
            wy_t = constp.tile([128, 128], F16)
            nc.scalar.dma_start(out=wy_t[:], in_=wy[:])
            id_t = constp.tile([128, 128], F16)
            nc.scalar.dma_start(out=id_t[:], in_=idm[:])

            n_waves = PAIRS // WAVE
            s_tiles = {}
            v_tiles = {}
            stage_tiles = {}
            ysl_tiles = {}

            def stage_load(p):
                stage = stagep.tile([15, T, FREE], F16, name=f"stage{p}",
                                    tag="stage")
                stage_tiles[p] = stage
                (nc.sync if p % 2 == 0 else nc.gpsimd).dma_start(
                    out=stage[:, :, :],
                    in_=xstage[15 * p:15 * (p + 1), :].rearrange(
                        "r (t f) -> r t f", t=T),
                )

            def emit_y_phase(p, thalf):
                """y matmuls for pair p, t in [4*thalf, 4*thalf+4)."""
                strat = STRATS[p % len(STRATS)]
                wyo = 64 if strat == "B" else 0
                if thalf == 0:
                    ysl_tiles[p] = ypool.tile([128, 2 * FREE], F32,
                                              name=f"ysl{p}", tag="ysl")
                ysl = ysl_tiles[p]
                for t in range(4 * thalf, 4 * thalf + 4):
                    s = s_tiles.pop((p, t))
                    prow = 32 * (t % 4)
                    cbase = (t // 4) * FREE
                    outsl = ysl[prow:prow + 32, cbase:cbase + FREE]
                    tp = (0, prow)
                    if W2_SPLIT:
                        nc.tensor.matmul(outsl, lhsT=wy_t[:, wyo:wyo + 32],
                                         rhs=s[:, :], start=True, stop=False,
                                         tile_position=tp)
                        nc.tensor.matmul(outsl, lhsT=wy_t[:, wyo + 32:wyo + 64],
                                         rhs=s[:, :], start=False, stop=True,
                                         tile_position=tp)
                    else:
                        nc.tensor.matmul(outsl, lhsT=wy_t[:, wyo:wyo + 32],
                                         rhs=s[:, :], start=True, stop=True,
                                         tile_position=tp)
                if thalf == 1:
                    emit_evac(p, ysl)

            def emit_evac(p, ysl):
                strat = STRATS[p % len(STRATS)]
                y_sb = ysbp.tile([128, 2 * FREE], F32, name=f"ysb{p}",
                                 tag="ysb")
                bcol = 0 if strat == "A" else 1
                if p in EVAC_ACT_PAIRS:
                    nc.scalar.activation(
                        out=y_sb[:], in_=ysl[:],
                        func=mybir.ActivationFunctionType.Identity,
                        bias=b2_t[:, bcol:bcol + 1], scale=1.0,
                    )
                else:
                    nc.vector.tensor_scalar(
                        out=y_sb[:], in0=ysl[:], scalar1=b2_t[:, bcol:bcol + 1],
                        scalar2=None, op0=mybir.AluOpType.add,
                    )
                ypext = y_sb.ap[0][0]
                for k in range(4):
                    # t = k and t = k+4 share src partitions {32k, 32k+1}
                    src_ap = bass.AP(
                        tensor=y_sb.tensor,
                        offset=y_sb.offset + 32 * k * ypext,
                        ap=[[ypext, 2], [FREE, 2], [1, FREE]],
                    )
                    dst_ap = bass.AP(
                        tensor=yout,
                        offset=(k * B + 2 * p) * DLOC,
                        ap=[[DLOC, 2], [4 * B * DLOC, 2], [1, DLOC]],
                    )
                    eng = nc.gpsimd if k % 2 == 0 else nc.sync
                    eng.dma_start(out=dst_ap, in_=src_ap)

            for p in range(min(WAVE, PAIRS)):
                stage_load(p)

            yq = []           # ready y-units (pair, thalf)
            loaded = min(WAVE, PAIRS)
            waves = []
            q = 0
            while q < PAIRS:
                waves.append(list(range(q, min(q + WAVE, PAIRS))))
                q += WAVE
            for wave in waves:
                for t in range(T):
                    # drain ready y work into this slot (up to 2 units)
                    for _ in range(2):
                        if yq:
                            pp, hh = yq.pop(0)
                            emit_y_phase(pp, hh)
                    # --- h accumulate ---
                    for p in wave:
                        strat = STRATS[p % len(STRATS)]
                        if t == 0:
                            v_tiles[p] = vpool.tile([128, FREE], F32,
                                                    name=f"v{p}", tag="v")
                        whv = wh_t[:] if (strat == "A" or t == 0) else wh_r[:]
                        nc.tensor.matmul(
                            v_tiles[p][:, :], lhsT=whv,
                            rhs=stage_tiles[p][:, t, :],
                            start=(t == 0), stop=True, skip_group_check=True,
                        )
                    # --- spike indicator ---
                    for p in wave:
                        strat = STRATS[p % len(STRATS)]
                        s = spool.tile([128, FREE], F16, name=f"s{p}_{t}",
                                       tag="s")
                        s_tiles[(p, t)] = s
                        v = v_tiles[p]
                        if strat == "A":
                            nc.vector.tensor_scalar(
                                out=s[:], in0=v[:], scalar1=1.0, scalar2=None,
                                op0=mybir.AluOpType.is_ge,
                            )
                        else:
                            nc.scalar.activation(
                                out=s[:], in_=v[:],
                                func=mybir.ActivationFunctionType.Sign,
                                bias=b2_t[:, 2:3], scale=1.0,
                            )
                        if t == 3:
                            yq.append((p, 0))
                        elif t == T - 1:
                            yq.append((p, 1))
                    # --- reset inject ---
                    if t < T - 1:
                        for p in wave:
                            strat = STRATS[p % len(STRATS)]
                            s, v = s_tiles[(p, t)], v_tiles[p]
                            if strat == "A":
                                nc.vector.tensor_tensor(
                                    out=v[:], in0=v[:], in1=s[:],
                                    op=mybir.AluOpType.subtract,
                                )
                            elif p in INJ_DVE_PAIRS:
                                nc.vector.scalar_tensor_tensor(
                                    out=v[:], in0=s[:], scalar=-0.5, in1=v[:],
                                    op0=mybir.AluOpType.mult,
                                    op1=mybir.AluOpType.add,
                                )
                            else:
                                nc.tensor.matmul(
                                    v[:, :], lhsT=id_t[:, :], rhs=s[:, :],
                                    start=False, stop=True,
                                    skip_group_check=True,
                                )
                    if loaded < PAIRS:
                        stage_load(loaded)
                        loaded += 1
            # tail: drain remaining y work
            for pp, hh in yq:
                emit_y_phase(pp, hh)
    nc.compile()
    return nc


def _prep_inputs(x, W1, b1, W2, b2, thr):
    a = W1[:, 0].astype(np.float32) / thr
    c = b1.astype(np.float32) / thr
    ah, am, al = _split3(a)
    w2p = thr * W2[0].astype(np.float32)

    wh = np.zeros((30, 128), np.float16)
    for base, cvals in ((0, c), (15, c - 0.5)):
        ch, cm, cl = _split3(cvals)
        for g in range(2):
            cols = slice(64 * g, 64 * (g + 1))
            rows = base + 6 * g
            wh[rows + 0, cols] = ah   # pairs x_hi
            wh[rows + 1, cols] = am   # pairs x_hi
            wh[rows + 2, cols] = al   # pairs x_hi
            wh[rows + 3, cols] = ah   # pairs x_mid
            wh[rows + 4, cols] = am   # pairs x_mid
            wh[rows + 5, cols] = ah   # pairs x_lo
        wh[base + 12, :64] = wh[base + 12, 64:] = ch
        wh[base + 13, :64] = wh[base + 13, 64:] = cm
        wh[base + 14, :64] = wh[base + 14, 64:] = cl

    wy = np.zeros((128, 128), np.float16)
    for varbase, scale in ((0, 1.0), (64, 0.5)):
        vv = (scale * w2p).astype(np.float32)
        vh = vv.astype(np.float16)
        vl = (vv - vh.astype(np.float32)).astype(np.float16)
        for g in range(2):
            rows = slice(64 * g, 64 * (g + 1))
            wy[rows, varbase + g] = vh          # hi block cols 0,1
            wy[rows, varbase + 32 + g] = vl     # lo block cols 32,33
    idm = (-0.5 * np.eye(128, dtype=np.float32)).astype(np.float16)
    b2c = np.zeros((128, 4), np.float32)
    b2c[:, 0] = np.float32(b2[0])
    b2c[:, 1] = np.float32(b2[0] + 0.5 * w2p.sum(dtype=np.float32))
    b2c[:, 2] = np.float32(-1.0)
    return wh, wy, idm, b2c


def _build_xstage(x):
    """Per-core staged x layout: [NCORES][PAIRS*15, T*FREE] fp16.

    rows 0-2: x_hi grpA (x3), 3-4: x_mid A, 5: x_lo A, 6-11: same grpB,
    12-14: ones.  free = t*FREE + d,  b = 2p + g.
    """
    xh, xm_, xl_ = _split3(x)                       # [T*B, D]
    out = np.empty((NCORES, PAIRS * 15, T * FREE), np.float16)
    rowmap = [(0, 0), (0, 0), (0, 0), (1, 0), (1, 0), (2, 0),
              (0, 1), (0, 1), (0, 1), (1, 1), (1, 1), (2, 1)]
    parts = (xh, xm_, xl_)
    for core in range(NCORES):
        dsl = slice(core * DLOC, (core + 1) * DLOC)
        for p in range(PAIRS):
            for r, (pi, g) in enumerate(rowmap):
                arr = parts[pi].reshape(T, B, D)[:, 2 * p + g, dsl]
                out[core, 15 * p + r] = arr.reshape(-1)
            out[core, 15 * p + 12:15 * p + 15] = np.float16(1.0)
    return out


def kernel(x, W1, b1, W2, b2, threshold, T: int = T, **_unused):
    x = np.asarray(x, np.float32)
    W1 = np.asarray(W1, np.float32)
    b1 = np.asarray(b1, np.float32)
    W2 = np.asarray(W2, np.float32)
    b2 = np.atleast_1d(np.asarray(b2, np.float32))
    thr = float(np.asarray(threshold))
    assert x.shape == (128, 4096), x.shape

    key = (STRATS, tuple(sorted(INJ_DVE_PAIRS)), W2_SPLIT, WAVE)
    if key not in _prog_cache:
        _prog_cache[key] = _build_program()
    nc = _prog_cache[key]

    wh, wy, idm, b2c = _prep_inputs(x, W1, b1, W2, b2, thr)
    xstage = _build_xstage(x)

    in_maps = []
    for core in range(NCORES):
        in_maps.append({
            "xstage": xstage[core],
            "wh": wh,
            "wy": wy,
            "idm": idm,
            "b2c": b2c,
        })
    global LAST_EXEC_NS
    kw = {}
    if TRACE:
        _ensure_ntff_hook()
        kw = dict(trace=True, trace_cores=[0])
    res = run_bass_kernel_spmd(nc, in_maps, core_ids=list(range(NCORES)), **kw)
    if TRACE:
        LAST_EXEC_NS = res.exec_time_ns
    out = np.concatenate([res.results[i]["y"] for i in range(NCORES)], axis=1)
    return out.astype(np.float32)
